# revision 1
# baseline (speedup 1.0000x reference)
"""Trainium2 Bass kernel for nn_DConv (diffusion graph conv, K=2, 2 supports).

Contract: kernel(**inputs) takes FULL unsharded inputs (inputs [B,N,D] f32,
adj_vals [E] f32, rows/cols [E] int, weights [D*M,OUT] f32, biases [1,OUT]
f32) and returns the FULL output [B, N, OUT] f32.

Strategy (data-parallel over batch, per the sharding hint):
 - Each of the 8 cores handles B/8 batches: x layout [N, D*Bl] (col = d*Bl+b).
 - Host builds the two normalized supports (vals1,rows->cols / vals2,cols->rows),
   sorts each edge list by destination into 128-node blocks, pads each block's
   edge segment to a multiple of 128 "slots".
 - Device, per spmm: dma_gather (bf16, 512B rows) fetches x[src] per slot;
   a per-chunk [128,128] selection matrix Sel[e, dst_local] = v_e (built on
   the vector engine as (iota==dst)*v) reduces each chunk into PSUM via
   TensorE: y_block += Sel^T @ Z. Eviction is a plain PSUM->bf16 copy; the
   Chebyshev recurrence (x2 = 2*S*x1 - x0) is folded into the projection
   weights on the host, so the 4 spmms produce raw S-products only:
     A1 = S1 X0, R2 = S1 A1, B1 = S2 A1, R4 = S2 B1
   out = X0(W0-W2) + A1(W1-W4) + R2(2 W2) + B1 W3 + R4(2 W4) + bias.
 - Projection: DMA-transpose loads X_m^T tiles, TensorE contracts against a
   host-built block-diagonal W~ [1280, OUT*Bl].
"""
import os
import sys
import numpy as np
import ml_dtypes

for _p in ('/opt/trn_rl_repo', '/root/.axon_site/_ro/trn_rl_repo'):
    if os.path.isdir(_p) and _p not in sys.path:
        sys.path.append(_p)

import concourse.bass as bass
import concourse.mybir as mybir
import concourse.tile as tile
from concourse import bacc
from concourse.bass_utils import run_bass_kernel_spmd

BF16 = ml_dtypes.bfloat16
P = 128
NCORES = 8


# ---------------------------------------------------------------- host prep

def _build_support(vals, src, dst, n_nodes):
    """Sort edges by dst, pad each 128-node block segment to a multiple of
    128 slots. Returns slot arrays + chunk metadata."""
    nb = n_nodes // P
    order = np.argsort(dst, kind='stable')
    s_src = src[order]
    s_dst = dst[order]
    s_v = vals[order]
    blk = (s_dst // P).astype(np.int64)
    cnt = np.bincount(blk, minlength=nb)

    src_parts, dstl_parts, v_parts = [], [], []
    chunk_block = []
    pos = 0
    for b in range(nb):
        c = int(cnt[b])
        nchunk = max(1, -(-c // P))
        pad = nchunk * P - c
        src_parts.append(s_src[pos:pos + c])
        dstl_parts.append(s_dst[pos:pos + c] - b * P)
        v_parts.append(s_v[pos:pos + c])
        if pad:
            src_parts.append(np.zeros(pad, s_src.dtype))
            dstl_parts.append(np.zeros(pad, s_dst.dtype))
            v_parts.append(np.zeros(pad, np.float32))
        chunk_block += [b] * nchunk
        pos += c

    slot_src = np.concatenate(src_parts).astype(np.int16)
    slot_dstl = np.concatenate(dstl_parts).astype(np.float32)
    slot_v = np.concatenate(v_parts).astype(np.float32)
    n_chunks = len(chunk_block)

    # slot-major [128, n_chunks]: arr[p, c] = val[c*128 + p]
    dst_t = np.ascontiguousarray(slot_dstl.reshape(n_chunks, P).T)
    v_t = np.ascontiguousarray(slot_v.reshape(n_chunks, P).T)

    # wrapped idx layout [128, n_slots/16]: tile[p, j] = idx[j*16 + p%16]
    idx = slot_src.reshape(-1, 16).T  # [16, n_slots/16]
    idx_w = np.ascontiguousarray(np.tile(idx, (8, 1)))

    # chunk -> (block, first, last)
    chunk_block = np.asarray(chunk_block)
    first = np.ones(n_chunks, bool)
    first[1:] = chunk_block[1:] != chunk_block[:-1]
    last = np.ones(n_chunks, bool)
    last[:-1] = chunk_block[:-1] != chunk_block[1:]
    return dict(idx_w=idx_w, dst_t=dst_t, v_t=v_t,
                chunk_block=chunk_block, first=first, last=last,
                n_chunks=n_chunks)


def preprocess(adj_vals, rows, cols, n_nodes):
    drow = np.zeros(n_nodes, np.float32)
    np.add.at(drow, rows, adj_vals)
    dcol = np.zeros(n_nodes, np.float32)
    np.add.at(dcol, cols, adj_vals)
    inv_drow = np.where(drow > 0, 1.0 / drow, 0.0).astype(np.float32)
    inv_dcol = np.where(dcol > 0, 1.0 / dcol, 0.0).astype(np.float32)
    vals1 = (adj_vals * inv_drow[rows]).astype(np.float32)
    vals2 = (adj_vals * inv_dcol[cols]).astype(np.float32)
    s1 = _build_support(vals1, rows, cols, n_nodes)
    s2 = _build_support(vals2, cols, rows, n_nodes)
    return s1, s2


def build_wtilde(weights, d_in, n_mat, out_dim, bl):
    """W~ [5*d_in*bl, out_dim*bl] bf16 with recurrence folded in.
    Row r = m*(d_in*bl) + (d*bl + b); col = o*bl + b."""
    W = weights.reshape(d_in, n_mat, out_dim)
    C = [W[:, 0] - W[:, 2], W[:, 1] - W[:, 4], 2.0 * W[:, 2], W[:, 3], 2.0 * W[:, 4]]
    F = d_in * bl
    Wt = np.zeros((5 * F, out_dim * bl), np.float32)
    for m in range(5):
        for d in range(d_in):
            for b in range(bl):
                Wt[m * F + d * bl + b, b::bl] = C[m][d]
    return Wt.astype(BF16)


# ---------------------------------------------------------------- program

def build_program(n_nodes, feat, out_feat, sup_metas, call_chunks=64, selg=8):
    """Build the per-core Bass program. sup_metas = (s1, s2) chunk metadata
    (only n_chunks/chunk_block/first/last are used — the program layout
    depends on them)."""
    ob = 256  # out_dim * bl
    nt = n_nodes // P  # projection node tiles
    n_wchunks = 5 * feat // P

    nc = bacc.Bacc("TRN2", target_bir_lowering=False, debug=False,
                   num_devices=NCORES)
    dt = mybir.dt

    x0 = nc.dram_tensor("x0", [n_nodes, feat], dt.bfloat16, kind="ExternalInput")
    iota_in = nc.dram_tensor("iota", [P, P], dt.float32, kind="ExternalInput")
    wt_in = nc.dram_tensor("wt", [5 * feat, ob], dt.bfloat16, kind="ExternalInput")
    bias_in = nc.dram_tensor("bias", [P, ob], dt.float32, kind="ExternalInput")

    sup_t = []
    for i, s in enumerate(sup_metas):
        n_slots = s['n_chunks'] * P
        sup_t.append(dict(
            idx=nc.dram_tensor(f"idx{i}", [P, n_slots // 16], dt.int16,
                               kind="ExternalInput"),
            dst=nc.dram_tensor(f"dst{i}", [P, s['n_chunks']], dt.float32,
                               kind="ExternalInput"),
            v=nc.dram_tensor(f"v{i}", [P, s['n_chunks']], dt.float32,
                             kind="ExternalInput"),
        ))

    A1 = nc.dram_tensor("A1", [n_nodes, feat], dt.bfloat16, kind="Internal")
    R2 = nc.dram_tensor("R2", [n_nodes, feat], dt.bfloat16, kind="Internal")
    B1 = nc.dram_tensor("B1", [n_nodes, feat], dt.bfloat16, kind="Internal")
    R4 = nc.dram_tensor("R4", [n_nodes, feat], dt.bfloat16, kind="Internal")
    out = nc.dram_tensor("out", [n_nodes, ob], dt.float32, kind="ExternalOutput")

    with tile.TileContext(nc) as tc:
        with (
            tc.tile_pool(name="const", bufs=1) as cpool,
            tc.tile_pool(name="z", bufs=2) as zpool,
            tc.tile_pool(name="idx", bufs=2) as ipool,
            tc.tile_pool(name="dv", bufs=2) as dvpool,
            tc.tile_pool(name="sel", bufs=2) as selpool,
            tc.tile_pool(name="ev", bufs=4) as evpool,
            tc.tile_pool(name="lhs", bufs=2) as lpool,
            tc.tile_pool(name="po", bufs=2) as opool,
            tc.tile_pool(name="ps", bufs=4, space="PSUM") as pspool,
            tc.tile_pool(name="pso", bufs=2, space="PSUM") as psopool,
        ):
            iota_sb = cpool.tile([P, P], dt.float32)
            nc.sync.dma_start(iota_sb[:], iota_in[:, :])
            wt_sb = cpool.tile([P, n_wchunks, ob], dt.bfloat16)
            nc.sync.dma_start(
                wt_sb[:],
                wt_in[:, :].rearrange("(k p) o -> p k o", p=P))
            bias_sb = cpool.tile([P, ob], dt.float32)
            nc.sync.dma_start(bias_sb[:], bias_in[:, :])

            def emit_spmm(sup, st, xsrc, ydst):
                n_chunks = sup['n_chunks']
                cb = sup['chunk_block']
                first = sup['first']
                last = sup['last']
                ps = None
                for c0 in range(0, n_chunks, call_chunks):
                    ncall = min(call_chunks, n_chunks - c0)
                    nidx = ncall * P
                    idx_t = ipool.tile([P, call_chunks * 8], dt.int16, tag="idx")
                    nc.sync.dma_start(
                        idx_t[:, :ncall * 8],
                        st['idx'][:, c0 * 8:(c0 + ncall) * 8])
                    dst_t = dvpool.tile([P, call_chunks], dt.float32, tag="dst")
                    nc.sync.dma_start(dst_t[:, :ncall],
                                      st['dst'][:, c0:c0 + ncall])
                    v_t = dvpool.tile([P, call_chunks], dt.float32, tag="v")
                    nc.sync.dma_start(v_t[:, :ncall],
                                      st['v'][:, c0:c0 + ncall])
                    z_t = zpool.tile([P, call_chunks, feat], dt.bfloat16, tag="z")
                    nc.gpsimd.dma_gather(
                        z_t[:, :ncall, :], xsrc[:, :], idx_t[:, :ncall * 8],
                        nidx, nidx, feat, single_packet=False)
                    sel_t = selpool.tile([P, call_chunks, P], dt.bfloat16,
                                         tag="sel")
                    for g0 in range(0, ncall, selg):
                        ng = min(selg, ncall - g0)
                        sel_sl = sel_t[:, g0:g0 + ng, :]
                        nc.vector.tensor_tensor(
                            out=sel_sl,
                            in0=iota_sb[:][:, None, :].to_broadcast([P, ng, P]),
                            in1=dst_t[:, g0:g0 + ng, None].to_broadcast([P, ng, P]),
                            op=mybir.AluOpType.is_equal)
                        nc.vector.tensor_tensor(
                            out=sel_sl,
                            in0=sel_sl,
                            in1=v_t[:, g0:g0 + ng, None].to_broadcast([P, ng, P]),
                            op=mybir.AluOpType.mult)
                    for cl in range(ncall):
                        c = c0 + cl
                        if first[c]:
                            ps = pspool.tile([P, feat], dt.float32, tag="ps")
                        nc.tensor.matmul(
                            out=ps[:],
                            lhsT=sel_t[:, cl, :],
                            rhs=z_t[:, cl, :],
                            start=bool(first[c]),
                            stop=bool(last[c]),
                        )
                        if last[c]:
                            b = cb[c]
                            y_sb = evpool.tile([P, feat], dt.bfloat16, tag="y")
                            nc.vector.tensor_copy(out=y_sb[:], in_=ps[:])
                            nc.sync.dma_start(
                                ydst[b * P:(b + 1) * P, :], y_sb[:])

            emit_spmm(sup_metas[0], sup_t[0], x0, A1)
            emit_spmm(sup_metas[0], sup_t[0], A1, R2)
            emit_spmm(sup_metas[1], sup_t[1], A1, B1)
            emit_spmm(sup_metas[1], sup_t[1], B1, R4)

            # projection
            xs = [x0, A1, R2, B1, R4]
            for t in range(nt):
                rows = slice(t * P, (t + 1) * P)
                pso = psopool.tile([P, ob], dt.float32, tag="pso")
                for k in range(n_wchunks):
                    m, h = divmod(k, feat // P)
                    lhsT = lpool.tile([P, P], dt.bfloat16, tag="lhsT")
                    nc.sync.dma_start_transpose(
                        lhsT[:], xs[m][rows, h * P:(h + 1) * P])
                    nc.tensor.matmul(
                        out=pso[:],
                        lhsT=lhsT[:],
                        rhs=wt_sb[:, k, :],
                        start=(k == 0),
                        stop=(k == n_wchunks - 1),
                    )
                o_sb = opool.tile([P, ob], dt.float32, tag="osb")
                nc.vector.tensor_tensor(out=o_sb[:], in0=pso[:],
                                        in1=bias_sb[:],
                                        op=mybir.AluOpType.add)
                nc.sync.dma_start(out[rows, :], o_sb[:])

    nc.compile()
    return nc


# ---------------------------------------------------------------- entry

def _make_core_inputs(core, inputs_f32, s1, s2, wt, bias_rep, n_nodes, d_in):
    bl = inputs_f32.shape[0] // NCORES
    x0 = np.ascontiguousarray(
        inputs_f32[core * bl:(core + 1) * bl]
        .transpose(1, 2, 0).reshape(n_nodes, d_in * bl)).astype(BF16)
    iota = np.tile(np.arange(P, dtype=np.float32)[None, :], (P, 1))
    return dict(
        x0=x0, iota=iota, wt=wt, bias=bias_rep,
        idx0=s1['idx_w'], dst0=s1['dst_t'], v0=s1['v_t'],
        idx1=s2['idx_w'], dst1=s2['dst_t'], v1=s2['v_t'],
    )


def kernel(**inputs):
    inputs_f32 = np.asarray(inputs['inputs'], dtype=np.float32)
    adj_vals = np.asarray(inputs['adj_vals'], dtype=np.float32)
    rows = np.asarray(inputs['rows']).astype(np.int64)
    cols = np.asarray(inputs['cols']).astype(np.int64)
    weights = np.asarray(inputs['weights'], dtype=np.float32)
    biases = np.asarray(inputs['biases'], dtype=np.float32)

    b_total, n_nodes, d_in = inputs_f32.shape
    out_dim = weights.shape[1]
    n_mat = weights.shape[0] // d_in
    bl = b_total // NCORES
    assert n_mat == 5, "kernel is specialized for K=2 (M=5)"

    s1, s2 = preprocess(adj_vals, rows, cols, n_nodes)
    wt = build_wtilde(weights, d_in, n_mat, out_dim, bl)
    bias_rep = np.zeros((P, out_dim * bl), np.float32)
    for o in range(out_dim):
        bias_rep[:, o * bl:(o + 1) * bl] = biases[0, o]

    nc = build_program(n_nodes, d_in * bl, out_dim, (s1, s2))

    in_maps = [
        _make_core_inputs(c, inputs_f32, s1, s2, wt, bias_rep, n_nodes, d_in)
        for c in range(NCORES)
    ]
    res = run_bass_kernel_spmd(nc, in_maps, core_ids=list(range(NCORES)))

    out = np.zeros((b_total, n_nodes, out_dim), np.float32)
    for c in range(NCORES):
        oc = res.results[c]['out']  # [n_nodes, out*bl], col = o*bl + b
        out[c * bl:(c + 1) * bl] = (
            oc.reshape(n_nodes, out_dim, bl).transpose(2, 0, 1))
    return out

